# revision 1
# baseline (speedup 1.0000x reference)
"""AdaptiveProductHead retrieval scoring kernel for 8 TRN2 NeuronCores.

Strategy (corpus sharding, no collectives):
  - x_c [65536, 768] is split 8 ways along the corpus dim; each core scores
    its [512, 8192] block; the host concatenates.
  - Host pre-transposes x_c/x_q to feature-major bf16 so the feature
    contraction (768) lands on SBUF partitions for matmuls.
  - Algebraic reformulation (validated vs reference to 2e-6 in f64):
      * dist_e term: S_e = 2*w0*(q_e . c_e)   (w0 folded into query rows)
      * dist_s = arccos(x)^2 = (2*arctan(sqrt((1-x)/(1+x))))^2,
        computed from p = 1+x via one reciprocal.
      * dist_h = (2*artanh(t))^2 with t^2 = Delta/den.  With the rank-1
        scaling z = Delta * (1/(1-xn)) * (1/(1-yn)) we get den_scaled = z+1,
        so dist_h = (2*arcsinh(sqrt(z)))^2 = (2*ln(sqrt(z)+sqrt(z+1)))^2 --
        no per-element division at all.
  - Engines: PE does projections + 3 score matmuls (bf16, row-strip packed);
    ACT does Sqrt/Tanh/Arctan/Ln (fp32); DVE does reciprocal + squares +
    combines (bf16 2x where possible); GPSIMD adds sqrt(z)+sqrt(z+1).
"""

import os
import sys
from contextlib import ExitStack

import numpy as np

sys.path.insert(0, "/opt/trn_rl_repo")

import ml_dtypes  # noqa: E402

import concourse.bass as bass  # noqa: E402
import concourse.tile as tile  # noqa: E402
from concourse import bacc, mybir  # noqa: E402

F32 = mybir.dt.float32
BF16 = mybir.dt.bfloat16
AX = mybir.AxisListType
OP = mybir.AluOpType
AF = mybir.ActivationFunctionType

D = 768
NQ = 512
NC = 65536
NCORES = 8
GROUP = 2048          # corpus columns processed per staged group
ST = 1024             # PSUM supertile width for score matmuls


def _build(shard: int):
    """Build the single-core SPMD graph for a corpus shard of `shard` cols."""
    assert shard % GROUP == 0
    n_groups = shard // GROUP
    nc = bacc.Bacc("TRN2", target_bir_lowering=False, debug=False,
                   num_devices=NCORES)

    xct = nc.dram_tensor("xct", [D, shard], BF16, kind="ExternalInput").ap()
    xqt = nc.dram_tensor("xqt", [D, NQ], BF16, kind="ExternalInput").ap()
    wcat = nc.dram_tensor("wcat", [7 * 128, 96], BF16, kind="ExternalInput").ap()
    w2t = nc.dram_tensor("w2t", [33, 4], BF16, kind="ExternalInput").ap()
    ident = nc.dram_tensor("ident", [128, 128], BF16, kind="ExternalInput").ap()
    out = nc.dram_tensor("out", [NQ, shard], BF16, kind="ExternalOutput").ap()

    with tile.TileContext(nc) as tc:
        _body(tc, xct, xqt, wcat, w2t, ident, out, shard, n_groups)
    nc.compile()
    return nc


def _body(tc, xct, xqt, wcat, w2t, ident, out, shard, n_groups):
    nc = tc.nc
    ctx = ExitStack()
    with ctx:
        _body_inner(ctx, tc, nc, xct, xqt, wcat, w2t, ident, out, shard,
                    n_groups)


def _body_inner(ctx, tc, nc, xct, xqt, wcat, w2t, ident, out, shard, n_groups):
    sync = nc.sync
    from concourse.tile_rust import add_dep_helper
    _prev_act = [None]

    def act(out_ap, in_ap, func, **kw):
        inst = nc.scalar.activation(out_ap, in_ap, func, **kw)
        if _prev_act[0] is not None:
            add_dep_helper(inst.ins, _prev_act[0], sync=False,
                           reason="act table-set ordering")
        _prev_act[0] = inst.ins
        return inst
    # ---------------- pools ----------------
    consts = ctx.enter_context(tc.tile_pool(name="consts", bufs=1))
    qsmall = ctx.enter_context(tc.tile_pool(name="qsmall", bufs=1))
    # corpus prep
    xg_pool = ctx.enter_context(tc.tile_pool(name="xg", bufs=1))
    praw_ps_pool = ctx.enter_context(tc.tile_pool(name="praw_ps", bufs=1, space="PSUM"))
    praw_sb_pool = ctx.enter_context(tc.tile_pool(name="praw_sb", bufs=5))
    tp_ps_pool = ctx.enter_context(tc.tile_pool(name="tp_ps", bufs=1, space="PSUM"))
    norm_pool = ctx.enter_context(tc.tile_pool(name="norms", bufs=2))
    cproj_pool = ctx.enter_context(tc.tile_pool(name="cproj", bufs=2))
    cmaj_pool = ctx.enter_context(tc.tile_pool(name="cmaj", bufs=2))
    # main chain
    z_ps_pool = ctx.enter_context(tc.tile_pool(name="z_ps", bufs=2, space="PSUM"))
    p_ps_pool = ctx.enter_context(tc.tile_pool(name="p_ps", bufs=1, space="PSUM"))
    smbuf_pool = ctx.enter_context(tc.tile_pool(name="smbuf", bufs=1))
    abuf_pool = ctx.enter_context(tc.tile_pool(name="abuf", bufs=1))
    w2a2_pool = ctx.enter_context(tc.tile_pool(name="w2a2buf", bufs=1))
    tr32 = ctx.enter_context(tc.tile_pool(name="tr32", bufs=2))
    tr16 = ctx.enter_context(tc.tile_pool(name="tr16", bufs=2))
    outp = ctx.enter_context(tc.tile_pool(name="outp", bufs=2))

    # ---------------- constants ----------------
    wcat_sb = consts.tile([128, 7 * 96], BF16)
    for k in range(7):
        sync.dma_start(out=wcat_sb[:, k * 96:(k + 1) * 96],
                       in_=wcat[k * 128:(k + 1) * 128, :])
    w2t_sb = consts.tile([33, 4], BF16)
    sync.dma_start(out=w2t_sb[:], in_=w2t[:])
    ident_sb = consts.tile([128, 128], BF16)
    sync.dma_start(out=ident_sb[:], in_=ident[:])
    xqt_sb = tr32.tile([128, 6 * NQ], BF16, tag="st_t", name="xqt_sb")
    for k in range(6):
        sync.dma_start(out=xqt_sb[:, k * NQ:(k + 1) * NQ],
                       in_=xqt[k * 128:(k + 1) * 128, :])
    ones1 = consts.tile([1, 128], BF16)
    nc.vector.memset(ones1[:], 1.0)

    qrows_sb = consts.tile([128, NQ], BF16)     # score-matmul query rows
    h1t_sb = consts.tile([33, NQ], BF16)
    nc.vector.memset(h1t_sb[32:33, :], 1.0)
    # per-query scalars, one column per q-chunk
    bm1 = consts.tile([128, 1], F32)
    nc.vector.memset(bm1[:], -1.0)
    w0x2 = consts.tile([128, 4], F32)
    w1x4 = consts.tile([128, 4], F32)
    w2x4 = consts.tile([128, 4], F32)

    # ---------------- early DMA for group 0 (overlaps query prep) -------
    xg0_early = []
    for k in range(6):
        t = xg_pool.tile([128, 1024], BF16, tag=f"xg{k}", name=f"xg{k}e")
        sync.dma_start(out=t[:], in_=xct[k * 128:(k + 1) * 128, 0:1024])
        xg0_early.append(t)

    # ---------------- query prep (stage-batched across q-chunks) --------
    qp_all = qsmall.tile([128, 4 * 96], F32, tag="qpall")
    for qc in range(4):
        qp_ps = praw_ps_pool.tile([128, 256], F32, tag="praw", name="qp_ps")
        for k in range(6):
            nc.tensor.matmul(
                qp_ps[:, 0:96],
                lhsT=xqt_sb[:, k * NQ + qc * 128: k * NQ + (qc + 1) * 128],
                rhs=wcat_sb[:, k * 96:(k + 1) * 96],
                start=(k == 0), stop=False)
        nc.tensor.matmul(qp_ps[:, 0:96], lhsT=ones1[0:1, :],
                         rhs=wcat_sb[0:1, 6 * 96:7 * 96],
                         start=False, stop=True)
        nc.scalar.copy(qp_all[:, qc * 96:(qc + 1) * 96], qp_ps[:, 0:96])
    qp3 = qp_all[:].rearrange("p (q c) -> p q c", q=4)          # [128,4,96]
    sq_all = qsmall.tile([128, 256], F32, tag="qsq")
    nc.vector.tensor_mul(sq_all[:].rearrange("p (q c) -> p q c", q=4),
                         qp3[:, :, 0:64], qp3[:, :, 0:64])
    red = qsmall.tile([128, 16], F32, tag="qred")               # 4q x 4blk
    nc.vector.tensor_reduce(red[:],
                            sq_all[:].rearrange("p (b c) -> p b c", c=16),
                            axis=AX.X, op=OP.add)
    red3 = red[:].rearrange("p (q b) -> p q b", q=4)            # [128,4,4]
    ne2 = qsmall.tile([128, 4], F32, tag="qne2")
    nc.vector.tensor_add(ne2[:], red3[:, :, 0], red3[:, :, 1])
    rte = qsmall.tile([128, 4], F32, tag="qrte")
    act(rte[:], ne2[:], AF.Sqrt)                                # |e|
    rtsh = qsmall.tile([128, 8], F32, tag="qrtsh")              # (|s|,|h|) x4
    act(rtsh[:].rearrange("p (q b) -> p q b", q=4), red3[:, :, 2:4], AF.Sqrt)
    rtsh3 = rtsh[:].rearrange("p (q b) -> p q b", q=4)
    rce = qsmall.tile([128, 4], F32, tag="qrce")
    nc.vector.reciprocal_approx_fast(rce[:], rte[:])            # 1/|e|
    rcsh = qsmall.tile([128, 8], F32, tag="qrcsh")
    nc.vector.reciprocal_approx_fast(rcsh[:], rtsh[:])          # 1/|s|,1/|h|
    rcsh3 = rcsh[:].rearrange("p (q b) -> p q b", q=4)
    th = qsmall.tile([128, 4], F32, tag="qth")
    act(th[:], rtsh3[:, :, 1], AF.Tanh)                         # tanh(|h|)
    xn = qsmall.tile([128, 4], F32, tag="qxn")
    nc.vector.tensor_mul(xn[:], th[:], th[:])
    omx = qsmall.tile([128, 4], F32, tag="qomx")
    nc.vector.tensor_scalar(omx[:], xn[:], -1.0, 1.0, OP.mult, OP.add)
    ib = qsmall.tile([128, 4], F32, tag="qib")
    nc.vector.reciprocal_approx_fast(ib[:], omx[:])
    f_h = qsmall.tile([128, 4], F32, tag="qfh")
    nc.vector.tensor_mul(f_h[:], th[:], rcsh3[:, :, 1])         # tanh(n)/n
    fh2 = qsmall.tile([128, 4], F32, tag="qfh2")
    nc.vector.tensor_mul(fh2[:], f_h[:], ib[:])
    nc.vector.tensor_scalar_mul(fh2[:], fh2[:], -2.0)
    xnib = qsmall.tile([128, 4], F32, tag="qxnib")
    nc.vector.tensor_mul(xnib[:], xn[:], ib[:])
    # MLP: relu -> per-qc transpose -> matmul, then batched softplus
    h1_all = qsmall.tile([128, 128], BF16, tag="qh1")
    nc.vector.tensor_relu(h1_all[:].rearrange("p (q c) -> p q c", q=4),
                          qp3[:, :, 64:96])
    wpre = praw_ps_pool.tile([128, 256], F32, tag="praw", name="wpre")
    for qc in range(4):
        h1tp = tp_ps_pool.tile([128, 512], BF16, tag="tp", name="h1tp")
        nc.tensor.transpose(h1tp[0:32, 0:128],
                            h1_all[:, qc * 32:(qc + 1) * 32], ident_sb[:])
        nc.vector.tensor_copy(h1t_sb[0:32, qc * 128:(qc + 1) * 128],
                              h1tp[0:32, 0:128])
        nc.tensor.matmul(wpre[:, qc * 4:(qc + 1) * 4],
                         lhsT=h1t_sb[0:33, qc * 128:(qc + 1) * 128],
                         rhs=w2t_sb[:], start=True, stop=True,
                         tile_position=(0, 0))
    wex = qsmall.tile([128, 16], F32, tag="qwex")
    act(wex[:], wpre[:, 0:16], AF.Exp)
    wts = qsmall.tile([128, 16], F32, tag="qwts")
    act(wts[:], wex[:], AF.Ln, bias=1.0)                        # softplus
    wts3 = wts[:].rearrange("p (q c) -> p q c", q=4)
    nc.vector.tensor_scalar_mul(w0x2[:], wts3[:, :, 0], 2.0)
    nc.vector.tensor_scalar_mul(w1x4[:], wts3[:, :, 1], 4.0)
    nc.vector.tensor_scalar_mul(w2x4[:], wts3[:, :, 2], 4.0)
    ce = qsmall.tile([128, 4], F32, tag="qce")
    nc.vector.tensor_mul(ce[:], rce[:], w0x2[:])
    # assemble q_all (bf16) for all 4 chunks, then transpose into qrows
    qall = qsmall.tile([128, 512], BF16, tag="qall")
    nc.vector.memset(qall[:], 0.0)
    qa3 = qall[:].rearrange("p (q c) -> p q c", q=4)            # [128,4,128]
    def qbc(sc):
        return sc[:].unsqueeze(2)                               # [128,4,1]
    b0, b1 = bass.broadcast_tensor_aps(qp3[:, :, 0:32], qbc(ce))
    nc.vector.tensor_tensor(qa3[:, :, 0:32], b0, b1, OP.mult)
    b0, b1 = bass.broadcast_tensor_aps(qp3[:, :, 32:48], qbc(rcsh3[:, :, 0]))
    nc.vector.tensor_tensor(qa3[:, :, 32:48], b0, b1, OP.mult)
    b0, b1 = bass.broadcast_tensor_aps(qp3[:, :, 48:64], qbc(fh2))
    nc.vector.tensor_tensor(qa3[:, :, 64:80], b0, b1, OP.mult)
    nc.vector.memset(qa3[:, :, 48:49], 1.0)
    nc.vector.tensor_copy(qa3[:, :, 80:81], qbc(xnib))
    nc.vector.tensor_copy(qa3[:, :, 81:82], qbc(ib))
    for qc in range(4):
        qtp = tp_ps_pool.tile([128, 512], BF16, tag="tp", name="qtp")
        nc.tensor.transpose(qtp[:, 0:128],
                            qall[:, qc * 128:(qc + 1) * 128], ident_sb[:])
        nc.vector.tensor_copy(qrows_sb[:, qc * 128:(qc + 1) * 128],
                              qtp[:, 0:128])

    # ---------------- corpus prep (split into table-set phases) ----------
    def prep_a(g):
        """DMA + projection matmuls + squared-norm reduce + sqrt-set norms."""
        base = g * GROUP
        red_g = norm_pool.tile([128, 64], F32, tag="red")       # 16c x 4t
        praw_sbs = []
        for half in range(2):
            if g == 0 and half == 0:
                xg = xg0_early
            else:
                xg = []
                for k in range(6):
                    t = xg_pool.tile([128, 1024], BF16, tag=f"xg{k}", name=f"xg{k}")
                    sync.dma_start(
                        out=t[:],
                        in_=xct[k * 128:(k + 1) * 128,
                                base + half * 1024: base + (half + 1) * 1024])
                    xg.append(t)
            for pk in range(half * 2, half * 2 + 2):  # 2 packs per half
                praw_ps = praw_ps_pool.tile([128, 256], F32, tag="praw")
                for j in range(4):                    # chunk within pack
                    cc = (pk - half * 2) * 4 + j      # chunk within half
                    sl = praw_ps[:, j * 64:(j + 1) * 64]
                    for k in range(6):
                        nc.tensor.matmul(
                            sl, lhsT=xg[k][:, cc * 128:(cc + 1) * 128],
                            rhs=wcat_sb[:, k * 96: k * 96 + 64],
                            start=(k == 0), stop=False)
                    nc.tensor.matmul(sl, lhsT=ones1[0:1, :],
                                     rhs=wcat_sb[0:1, 6 * 96: 6 * 96 + 64],
                                     start=False, stop=True)
                praw_sb = praw_sb_pool.tile([128, 256], F32, tag="praw_sb")
                nc.vector.tensor_copy(praw_sb[:], praw_ps[:])
                praw_sbs.append(praw_sb)
                sq = praw_sb_pool.tile([128, 256], F32, tag="sqp", bufs=2)
                nc.vector.tensor_mul(sq[:], praw_sb[:], praw_sb[:])
                sq3 = sq[:].rearrange("p (c d) -> p c d", d=16)  # [128,16,16]
                nc.vector.tensor_reduce(red_g[:, pk * 16:(pk + 1) * 16],
                                        sq3, axis=AX.X, op=OP.add)
        red3 = red_g[:].rearrange("p (c t) -> p c t", t=4)      # [128,16,4]
        ne2 = norm_pool.tile([128, 16], F32, tag="ne2")
        nc.vector.tensor_add(ne2[:], red3[:, :, 0], red3[:, :, 1])
        rt_es = norm_pool.tile([128, 32], F32, tag="rt_es")     # |e| then |s|
        act(rt_es[:, 0:16], ne2[:], AF.Sqrt)
        act(rt_es[:, 16:32], red3[:, :, 2], AF.Sqrt)
        rth = norm_pool.tile([128, 16], F32, tag="rth")
        act(rth[:], red3[:, :, 3], AF.Sqrt)
        return dict(praw_sbs=praw_sbs, rt_es=rt_es, rth=rth)

    def prep_mid(pc):
        """Tanh of the hyperbolic norms (sigmoid table set)."""
        th = norm_pool.tile([128, 16], F32, tag="cth")
        act(th[:], pc["rth"][:], AF.Tanh)
        pc["th"] = th

    def prep_b(pc):
        """Scale factors + c-major assembly + PE transposes -> cproj."""
        rt_es, rth, th = pc["rt_es"], pc["rth"], pc["th"]
        cproj = cproj_pool.tile([128, GROUP], BF16, tag="cproj")
        fes = norm_pool.tile([128, 32], F32, tag="fes")
        nc.vector.reciprocal_approx_fast(fes[:], rt_es[:])      # 1/|e|, 1/|s|
        rcth = norm_pool.tile([128, 16], F32, tag="rcth")
        nc.vector.reciprocal_approx_fast(rcth[:], rth[:])
        f_h = norm_pool.tile([128, 16], F32, tag="cfh")
        nc.vector.tensor_mul(f_h[:], th[:], rcth[:])
        yn = norm_pool.tile([128, 16], F32, tag="cyn")
        nc.vector.tensor_mul(yn[:], th[:], th[:])
        omy = norm_pool.tile([128, 16], F32, tag="comy")
        nc.vector.tensor_scalar(omy[:], yn[:], -1.0, 1.0, OP.mult, OP.add)
        iy = norm_pool.tile([128, 16], F32, tag="ciy")
        nc.vector.reciprocal_approx_fast(iy[:], omy[:])
        fhiy = norm_pool.tile([128, 16], F32, tag="cfhiy")
        nc.vector.tensor_mul(fhiy[:], f_h[:], iy[:])
        yniy = norm_pool.tile([128, 16], F32, tag="cyniy")
        nc.vector.tensor_mul(yniy[:], yn[:], iy[:])
        for pk in range(4):
            praw_sb = pc["praw_sbs"][pk]
            p3 = praw_sb[:].rearrange("p (c f) -> p c f", c=4)  # [128,4,64]
            cm = cmaj_pool.tile([128, 512], BF16, tag="cmaj")
            nc.gpsimd.memset(cm[:], 0.0)
            c3 = cm[:].rearrange("p (c f) -> p c f", c=4)       # [128,4,128]
            def bc(sc):
                return sc[:, pk * 4:(pk + 1) * 4].unsqueeze(2)  # [128,4,1]
            b0, b1 = bass.broadcast_tensor_aps(p3[:, :, 0:32], bc(fes[:, 0:16]))
            nc.vector.tensor_tensor(c3[:, :, 0:32], b0, b1, OP.mult)
            b0, b1 = bass.broadcast_tensor_aps(p3[:, :, 32:48], bc(fes[:, 16:32]))
            nc.vector.tensor_tensor(c3[:, :, 32:48], b0, b1, OP.mult)
            b0, b1 = bass.broadcast_tensor_aps(p3[:, :, 48:64], bc(fhiy))
            nc.vector.tensor_tensor(c3[:, :, 64:80], b0, b1, OP.mult)
            nc.vector.memset(c3[:, :, 48:49], 1.0)
            nc.vector.tensor_copy(c3[:, :, 80:81], bc(iy))
            nc.vector.tensor_copy(c3[:, :, 81:82], bc(yniy))
            tp = tp_ps_pool.tile([128, 512], BF16, tag="tp")
            for j in range(4):
                nc.tensor.transpose(tp[:, j * 128:(j + 1) * 128],
                                    cm[:, j * 128:(j + 1) * 128], ident_sb[:])
            nc.vector.tensor_copy(
                cproj[:, pk * 512:(pk + 1) * 512].bitcast(mybir.dt.uint32),
                tp[:].bitcast(mybir.dt.uint32))
        return cproj

    # ---------------- main chain stages ----------------
    def sqrt_stage(g, cproj):
        units = [(st, qc) for st in range(GROUP // ST) for qc in range(4)]
        sm = [smbuf_pool.tile([128, GROUP], F32, tag=f"sm{qc}", name=f"sm{qc}") for qc in range(4)]
        ab32 = [abuf_pool.tile([128, GROUP], F32, tag=f"ab32_{qc}", name=f"ab32_{qc}") for qc in range(4)]
        for half in range(2):
            hqcs = [half * 2, half * 2 + 1]
            rects = []
            for qc in hqcs:
                rect = tr32.tile([128, GROUP], F32, tag="rect", bufs=2)
                for st in range(GROUP // ST):
                    lo = st * ST
                    z_ps = z_ps_pool.tile([128, ST], F32, tag="z")
                    p_ps = p_ps_pool.tile([128, ST], F32, tag="p", name="p_ps")
                    for h in range(2):
                        cs = slice(lo + h * 512, lo + (h + 1) * 512)
                        nc.tensor.matmul(z_ps[:, h * 512:(h + 1) * 512],
                                         lhsT=qrows_sb[64:82, qc * 128:(qc + 1) * 128],
                                         rhs=cproj[64:82, cs],
                                         tile_position=(64, 0), start=True, stop=True)
                        nc.tensor.matmul(p_ps[:, h * 512:(h + 1) * 512],
                                         lhsT=qrows_sb[32:49, qc * 128:(qc + 1) * 128],
                                         rhs=cproj[32:49, cs],
                                         tile_position=(32, 0), start=True, stop=True)
                    sl = slice(lo, lo + ST)
                    szt = tr32.tile([128, ST], F32, tag="szt")
                    s1zt = tr32.tile([128, ST], F32, tag="s1zt")
                    act(szt[:], z_ps[:], AF.Sqrt)
                    act(s1zt[:], z_ps[:], AF.Sqrt, bias=1.0)
                    nc.gpsimd.tensor_add(sm[qc][:, sl], szt[:], s1zt[:])
                    nc.vector.reciprocal_approx_fast(rect[:, sl], p_ps[:])
                rects.append(rect)
            for qc, rect in zip(hqcs, rects):
                st_t = tr32.tile([128, GROUP], F32, tag="st_t", bufs=2)
                act(st_t[:], rect[:], AF.Sqrt, bias=bm1[:], scale=2.0)
                sp_t = tr32.tile([128, GROUP], F32, tag="rect")
                nc.gpsimd.tensor_scalar_add(sp_t[:], st_t[:], 1.0)
                nc.vector.reciprocal_approx_fast(ab32[qc][:], sp_t[:])
        return sm, ab32

    def arctan_stage(g, ab32):
        ab = [abuf_pool.tile([128, GROUP], BF16, tag=f"ab{qc}", name=f"ab{qc}") for qc in range(4)]
        w2a2 = [w2a2_pool.tile([128, GROUP], BF16, tag=f"w2a2{qc}", name=f"w2a2{qc}") for qc in range(4)]
        for qc in range(4):
            act(ab[qc][:], ab32[qc][:], AF.Arctan, bias=1.0, scale=-2.0)
            asn = tr16.tile([128, GROUP], BF16, tag="asn", bufs=1)
            nc.vector.tensor_scalar_add(asn[:], ab[qc][:], float(np.pi / 4))
            t4 = tr16.tile([128, GROUP], BF16, tag="t4w", bufs=1)
            nc.vector.tensor_single_scalar(t4[:], asn[:],
                                           w2x4[:, qc:qc + 1], OP.mult)
            nc.gpsimd.tensor_mul(w2a2[qc][:], asn[:], t4[:])
        return w2a2

    def finish_group(g, cproj, sm, w2a2):
        base = g * GROUP
        units = [(st, qc) for st in range(GROUP // ST) for qc in range(4)]
        lh = [abuf_pool.tile([128, GROUP], BF16, tag=f"ab{qc}", name=f"lh{qc}")
              for qc in range(4)]
        for qc in range(4):
            act(lh[qc][:], sm[qc][:], AF.Ln)
        for st, qc in units:
            lo = st * ST
            sl = slice(lo, lo + ST)
            t4b = tr16.tile([128, ST], BF16, tag="t4b")
            nc.vector.tensor_single_scalar(t4b[:], lh[qc][:, sl],
                                           w1x4[:, qc:qc + 1], OP.mult)
            w1d2 = tr16.tile([128, ST], BF16, tag="w1d2")
            nc.gpsimd.tensor_mul(w1d2[:], lh[qc][:, sl], t4b[:])
            se_ps = p_ps_pool.tile([128, ST], F32, tag="p", name="se_ps")
            for h in range(2):
                cs = slice(lo + h * 512, lo + (h + 1) * 512)
                nc.tensor.matmul(se_ps[:, h * 512:(h + 1) * 512],
                                 lhsT=qrows_sb[0:32, qc * 128:(qc + 1) * 128],
                                 rhs=cproj[0:32, cs],
                                 tile_position=(0, 0), start=True, stop=True)
            acc = tr16.tile([128, ST], BF16, tag="acc")
            nc.vector.ln_bwd_dx(acc[:], se_ps[:], w1d2[:],
                                1.0, w0x2[:, qc:qc + 1], 1.0)
            ot = outp.tile([128, ST], BF16, tag="ot")
            nc.gpsimd.tensor_sub(ot[:], acc[:], w2a2[qc][:, sl])
            sync.dma_start(
                out=out[qc * 128:(qc + 1) * 128, base + lo: base + lo + ST],
                in_=ot[:])

    # ---------------- top-level schedule ----------------
    pc = prep_a(0)
    prep_mid(pc)
    cproj = prep_b(pc)
    for g in range(n_groups):
        sm, ab32 = sqrt_stage(g, cproj)                  # sqrt set
        pc_n = prep_a(g + 1) if g + 1 < n_groups else None   # sqrt set
        w2a2 = arctan_stage(g, ab32)                     # sigmoid set
        if pc_n is not None:
            prep_mid(pc_n)                               # sigmoid set
        cproj_n = prep_b(pc_n) if pc_n is not None else None
        finish_group(g, cproj, sm, w2a2)                 # ln set + combine
        cproj = cproj_n


# ---------------------------------------------------------------------------
# host-side entry point
# ---------------------------------------------------------------------------
_CACHE = {}
_LAST_RESULTS = None


def _prep_host_inputs(x_q, x_c, We, be, Wh, bh, Ws, bs, scale_h, W1, b1, W2, b2):
    bf = ml_dtypes.bfloat16
    sh = np.float32(scale_h)
    W_all = np.concatenate([We, Ws, sh * Wh, W1], axis=0).astype(np.float32)  # [96,768]
    b_all = np.concatenate([be, bs, sh * bh, b1], axis=0).astype(np.float32)  # [96]
    wcat = np.zeros((7 * 128, 96), np.float32)
    wcat[:768, :] = W_all.T
    wcat[768, :] = b_all
    w2t = np.zeros((33, 4), np.float32)
    w2t[:32, :3] = W2.T
    w2t[32, :3] = b2
    xqt = np.ascontiguousarray(x_q.T)
    xct = np.ascontiguousarray(x_c.T)
    return {
        "xqt": xqt.astype(bf),
        "xct": xct.astype(bf),
        "wcat": wcat.astype(bf),
        "w2t": w2t.astype(bf),
        "ident": np.eye(128, dtype=np.float32).astype(bf),
    }


def _ensure_trn_backend():
    """Make sure jax sees the 8 axon TRN cores even if another part of the
    process pinned jax to cpu first."""
    import jax
    try:
        devs = jax.devices()
        if len(devs) >= NCORES and devs[0].platform != "cpu":
            return
    except Exception:
        pass
    try:
        jax.config.update("jax_platforms", "axon")
        import jax.extend.backend
        jax.extend.backend.clear_backends()
        devs = jax.devices()
        assert len(devs) >= NCORES, devs
    except Exception as e:
        print("kernel: TRN backend re-init failed:", repr(e))


def kernel(x_q, x_c, We, be, Wh, bh, Ws, bs, scale_h, W1, b1, W2, b2):
    from concourse.bass_utils import run_bass_kernel_spmd

    _ensure_trn_backend()

    n_c = x_c.shape[0]
    shard = n_c // NCORES
    host = _prep_host_inputs(x_q, x_c, We, be, Wh, bh, Ws, bs, scale_h,
                             W1, b1, W2, b2)
    if shard not in _CACHE:
        _CACHE[shard] = _build(shard)
    nc = _CACHE[shard]
    in_maps = []
    for c in range(NCORES):
        m = {k: v for k, v in host.items() if k != "xct"}
        m["xct"] = np.ascontiguousarray(
            host["xct"][:, c * shard:(c + 1) * shard])
        in_maps.append(m)
    global _LAST_RESULTS
    trace = bool(int(os.environ.get("KBENCH_TRACE", "0")))
    res = run_bass_kernel_spmd(nc, in_maps, core_ids=list(range(NCORES)),
                               trace=trace)
    _LAST_RESULTS = res
    outs = [np.asarray(res.results[c]["out"]).astype(np.float32)
            for c in range(NCORES)]
    return np.concatenate(outs, axis=1)


if __name__ == "__main__":
    # smoke-build at small shard
    nc = _build(GROUP)
    print("build ok:", len(nc.m.functions[0].instructions) if hasattr(nc.m.functions[0], 'instructions') else "?")


def _pjrt_timed(nc, in_maps, iters):
    """Time `iters` back-to-back NEFF executions with device-resident inputs.
    Returns (t_total_seconds, per_iter_overhead_estimate)."""
    import time as _time

    import jax
    from jax.experimental.shard_map import shard_map
    from jax.sharding import Mesh, PartitionSpec, NamedSharding

    from concourse import bass2jax as b2j
    from concourse import mybir as _mb

    b2j.install_neuronx_cc_hook()
    partition_name = (nc.partition_id_tensor.name
                      if nc.partition_id_tensor else None)
    in_names, out_names, out_avals, zero_outs = [], [], [], []
    for alloc in nc.m.functions[0].allocations:
        if not isinstance(alloc, _mb.MemoryLocationSet):
            continue
        name = alloc.memorylocations[0].name
        if alloc.kind == "ExternalInput":
            if name != partition_name:
                in_names.append(name)
        elif alloc.kind == "ExternalOutput":
            shape = tuple(alloc.tensor_shape)
            dtype = _mb.dt.np(alloc.dtype)
            out_avals.append(jax.core.ShapedArray(shape, dtype))
            zero_outs.append(np.zeros(shape, dtype))
            out_names.append(name)
    n_params = len(in_names)
    n_outs = len(out_avals)
    in_names = in_names + out_names
    if partition_name is not None:
        in_names.append(partition_name)

    def _per_core(m):
        return [np.asarray(m[name]) for name in in_names[:n_params]]

    def _body(*args):
        operands = list(args)
        if partition_name is not None:
            operands.append(b2j.partition_id_tensor())
        outs = b2j._bass_exec_p.bind(
            *operands,
            out_avals=tuple(out_avals),
            in_names=tuple(in_names),
            out_names=tuple(out_names),
            lowering_input_output_aliases=(),
            sim_require_finite=True,
            sim_require_nnan=True,
            nc=nc,
        )
        return tuple(outs)

    n_cores = len(in_maps)
    devices = jax.devices()[:n_cores]
    mesh = Mesh(np.asarray(devices), ("core",))
    in_specs = (PartitionSpec("core"),) * (n_params + n_outs)
    out_specs = (PartitionSpec("core"),) * n_outs
    fn = jax.jit(shard_map(_body, mesh=mesh, in_specs=in_specs,
                           out_specs=out_specs, check_rep=False),
                 keep_unused=True)
    per_core = [_per_core(m) for m in in_maps]
    concat_in = [np.concatenate([per_core[c][i] for c in range(n_cores)], axis=0)
                 for i in range(n_params)]
    concat_zeros = [np.zeros((n_cores * z.shape[0], *z.shape[1:]), z.dtype)
                    for z in zero_outs]
    sh = NamedSharding(mesh, PartitionSpec("core"))
    dev_in = [jax.device_put(a, sh) for a in concat_in + concat_zeros]
    jax.block_until_ready(dev_in)
    outs = fn(*dev_in)          # compile + warm
    jax.block_until_ready(outs)
    t0 = _time.time()
    res = [fn(*dev_in) for _ in range(iters)]
    jax.block_until_ready(res)
    return _time.time() - t0


def time_exec(inp, iters=20):
    """Estimate per-NEFF-execution time by slope between iters and 1."""
    n_c = inp["x_c"].shape[0]
    shard = n_c // NCORES
    host = _prep_host_inputs(**inp)
    if shard not in _CACHE:
        _CACHE[shard] = _build(shard)
    nc = _CACHE[shard]
    in_maps = []
    for c in range(NCORES):
        m = {k: v for k, v in host.items() if k != "xct"}
        m["xct"] = np.ascontiguousarray(host["xct"][:, c * shard:(c + 1) * shard])
        in_maps.append(m)
    try:
        meas = []
        for _ in range(5):
            t1 = _pjrt_timed(nc, in_maps, 2)
            tn = _pjrt_timed(nc, in_maps, iters)
            meas.append((tn - t1) / (iters - 2) * 1e9)
        meas.sort()
        ns = meas[len(meas) // 2]
        print("slope samples (ns/iter):", [int(m) for m in meas])
        print("median slope %.0f ns/iter (includes ~0.3-1.1 ms/iter axon "
              "dispatch overhead; cost-model exec ~262 us)" % ns)
        return int(ns)
    except Exception as e:
        import traceback; traceback.print_exc()
        print("time_exec failed:", repr(e))
        return None

